# revision 57
# baseline (speedup 1.0000x reference)
"""GraphConv (DGL norm='both') + log_softmax on 8 Trainium2 NeuronCores.

Strategy (per sharding hint): partition nodes across the 8 cores by range.
  Launch A (per core): project its 12500-node slice m = (h @ W) * out_deg^-1/2
  in bf16 (PE bf16, PSUM f32 accumulate).
  Host: concatenate the 8 projected shards into a replicated gather table,
  viewed as PAIRED rows [50176, 128] bf16 so the table row stride is 256 B
  (DMA descriptor encoding granularity) while each gather moves only the
  needed 128-B half-row (the pair parity selects a 64-col offset).
  Launch B (per core): for its 12500 dst nodes, gather m[src] half-rows for
  all in-edges (dma_gather, edges pre-sorted by dst group), segment-sum via
  one-hot matmuls accumulating in PSUM, then norm/bias/log_softmax.

Degrees and the sorted/padded edge metadata are sharding-prep computed on the
host (numpy); all FLOPs on h/W/b/m (projection, normalization, aggregation,
softmax) run on device.
"""

import contextlib

import numpy as np
import ml_dtypes

import concourse.bass as bass
import concourse.bacc as bacc
import concourse.mybir as mybir
import concourse.tile as tile
from concourse.bass import AP
from concourse.bass_utils import run_bass_kernel_spmd

P = 128
N_NODES = 100000
N_EDGES = 3200000
IN_DIM = 256
OUT_DIM = 64
NCORES = 8
G = N_NODES // NCORES            # 12500 nodes per core
NG = (G + P - 1) // P            # 98 groups of 128 dst nodes (last has 84)
GPAD = NG * P                    # 12544
NPAIR = (NCORES * GPAD) // 2     # 50176 paired table rows
NT = 2                           # sub-tables (int16 index limit)
TROWS = NPAIR // NT              # 25088 rows per sub-table
NCLS = NT * 2                    # gather classes: (sub-table, parity)
ROUND_G = 8                      # max dst groups per gather round
# tapered round sizes: short first round starts compute early; short last
# rounds keep the post-gather drain chain small
ROUND_SIZES = [4] + [8] * 11 + [3, 2, 1]
assert sum(ROUND_SIZES) == NG
assert max(ROUND_SIZES) <= ROUND_G
FIN_EVERY = 4                    # rounds per log_softmax finalize batch
HBLK = 16                        # dst groups per hT load in launch A
PAD_LDST = 200.0                 # local-dst for padded edges (>127, exact bf16)

_f32 = mybir.dt.float32
_bf16 = mybir.dt.bfloat16
_i16 = mybir.dt.int16


def _expand_mid(ap, n):
    """[P, C] AP -> [P, n, C] AP repeating each partition row n times
    (middle broadcast keeps the last dim packed, so DVE 2x mode applies)."""
    (ps, pc), (cs, cc) = ap.ap[0], ap.ap[1]
    return AP(ap.tensor, ap.offset, [[ps, pc], [0, n], [cs, cc]])


def _expand_last(ap, n):
    """[P, C] AP -> [P, C, n] AP repeating each element n times along a new
    innermost (stride-0) dim."""
    (ps, pc), (cs, cc) = ap.ap[0], ap.ap[1]
    return AP(ap.tensor, ap.offset, [[ps, pc], [cs, cc], [0, n]])


def _dma_gather_half(eng, out_ap, in_ap, idxs_ap, num_idxs, elem_size,
                     elem_step, single_packet=False):
    """dma_gather with a sub-256B payload (row stride must stay 256B-aligned:
    elem_step * dtype_size % 256 == 0). Same IR as bass's dma_gather helper,
    minus its payload-granularity assert (the HW descriptor only constrains
    the stride; the payload is free-form)."""
    stride_bytes = elem_step * mybir.dt.size(in_ap.dtype)
    assert stride_bytes % 256 == 0
    assert in_ap.ap[0][0] == elem_step
    return eng.add_instruction(
        mybir.InstDMAGatherAnt(
            name=eng.bass.get_next_instruction_name(),
            ins=[*eng.lower_ap_dma(in_ap, for_custom_bir_dma=True),
                 eng.lower_ap(idxs_ap),
                 eng.lower_val_access(eng.to_reg(num_idxs))],
            outs=[eng.lower_ap(out_ap)],
            transpose=False,
            num_idxs=num_idxs,
            elem_size=elem_size,
            stride_bytes_256=stride_bytes // 256,
            gen_mode=0,
            single_packet=single_packet,
            queue_num=0,
            sbuf_tokens_per_rank=0,
            sbuf_free_dim_per_rank=0,
            sbuf_free_dim_pad_per_rank=0,
            sbuf_byte_offset=0,
        ))


# ---------------------------------------------------------------- launch A
def build_launch_a(repeat=1):
    nc = bacc.Bacc("TRN2", target_bir_lowering=False, debug=False,
                   num_devices=NCORES)
    hT = nc.dram_tensor("hT", [2, P, GPAD], _bf16, kind="ExternalInput")
    W = nc.dram_tensor("W", [2, P, OUT_DIM], _bf16, kind="ExternalInput")
    odeg = nc.dram_tensor("odeg", [P, NG], _f32, kind="ExternalInput")
    # partition-major projected features: m[p, g*64+f] = m_row(g*128+p, f)
    m = nc.dram_tensor("m", [P, NG * OUT_DIM], _bf16, kind="ExternalOutput")

    with tile.TileContext(nc) as tc:
        loop = tc.For_i(0, repeat, 1) if repeat > 1 \
            else contextlib.nullcontext()
        with loop, \
                tc.tile_pool(name="const", bufs=1) as cpool, \
                tc.tile_pool(name="hblk", bufs=4) as hpool, \
                tc.tile_pool(name="mstage", bufs=3) as mpool, \
                tc.tile_pool(name="psum", bufs=8, space="PSUM") as psum:
            w0 = cpool.tile([P, OUT_DIM], _bf16, tag="w0")
            w1 = cpool.tile([P, OUT_DIM], _bf16, tag="w1")
            nc.scalar.dma_start(out=w0[:], in_=W[0, :, :])
            nc.scalar.dma_start(out=w1[:], in_=W[1, :, :])

            dt_ = cpool.tile([P, NG], _f32, tag="deg")
            norm = cpool.tile([P, NG], _f32, tag="norm")
            nc.scalar.dma_start(out=dt_[:], in_=odeg[:, :])
            nc.vector.tensor_scalar_max(out=dt_[:], in0=dt_[:], scalar1=1.0)
            nc.vector.reciprocal(out=dt_[:], in_=dt_[:])
            nc.scalar.sqrt(out=norm[:], in_=dt_[:])

            for g0 in range(0, NG, HBLK):
                nb = min(HBLK, NG - g0)
                # both k-halves in one DMA: dram-side AP iterates (p, half,
                # node) to match the SBUF tile's (partition, half, node)
                lh = hpool.tile([P, 2, HBLK * P], _bf16, tag="lh")
                src = AP(hT[0, :, :].tensor, g0 * P,
                         [[GPAD, P], [P * GPAD, 2], [1, nb * P]])
                nc.sync.dma_start(out=lh[:, :, :nb * P], in_=src)
                ms = mpool.tile([P, HBLK, OUT_DIM], _bf16, tag="ms")
                # 8 groups per PSUM bank; norm scaling batched on DVE
                for j0 in range(0, nb, 8):
                    nj = min(8, nb - j0)
                    acc8 = psum.tile([P, 8, OUT_DIM], _f32, tag="acc8")
                    for j in range(j0, j0 + nj):
                        nc.tensor.matmul(acc8[:, j - j0, :],
                                         lh[:, 0, j * P:(j + 1) * P], w0[:],
                                         start=True, stop=False)
                        nc.tensor.matmul(acc8[:, j - j0, :],
                                         lh[:, 1, j * P:(j + 1) * P], w1[:],
                                         start=False, stop=True)
                    nc.vector.tensor_tensor(
                        out=ms[:, j0:j0 + nj, :], in0=acc8[:, :nj, :],
                        in1=_expand_last(norm[:, g0 + j0:g0 + j0 + nj],
                                         OUT_DIM),
                        op=mybir.AluOpType.mult)
                nc.gpsimd.dma_start(
                    out=m[:, g0 * OUT_DIM:(g0 + nb) * OUT_DIM],
                    in_=ms[:, :nb, :])
    nc.compile()
    return nc


# ---------------------------------------------------------------- launch B
def build_launch_b(meta, repeat=1):
    """meta["rounds"][i]:
      groups; q_numidx[NCLS]; q_choff[NCLS]; nch; idx_off; ch_off
      gldt: {g: (ldt_col_start, ngch)}   # ldst cols, group-major contiguous
      ggt:  {g: [gt_column, ...]}        # gather-tile column per oh chunk
    """
    nc = bacc.Bacc("TRN2", target_bir_lowering=False, debug=False,
                   num_devices=NCORES)
    tabs = [nc.dram_tensor(f"t{q}", [TROWS, 2 * OUT_DIM], _bf16,
                           kind="ExternalInput") for q in range(NT)]
    gidx = nc.dram_tensor("gidx", [P, meta["tot_idx_cols"]], _i16,
                          kind="ExternalInput")
    ldst = nc.dram_tensor("ldst", [P, meta["tot_chunks"]], _bf16,
                          kind="ExternalInput")
    max_gch = meta["max_gch"]
    ideg = nc.dram_tensor("ideg", [P, NG], _f32, kind="ExternalInput")
    brep = nc.dram_tensor("brep", [P, OUT_DIM], _f32, kind="ExternalInput")
    # partition-major output: out[p, g, f] = result(g*128+p, f)
    out = nc.dram_tensor("out", [P, NG, OUT_DIM], _bf16,
                         kind="ExternalOutput")

    with tile.TileContext(nc) as tc:
        loop = tc.For_i(0, repeat, 1) if repeat > 1 \
            else contextlib.nullcontext()
        with loop, \
                tc.tile_pool(name="const", bufs=1) as cpool, \
                tc.tile_pool(name="gath", bufs=2) as gpool, \
                tc.tile_pool(name="meta", bufs=5) as mpool, \
                tc.tile_pool(name="onehot", bufs=7) as opool, \
                tc.tile_pool(name="epi", bufs=4) as epool, \
                tc.tile_pool(name="yr", bufs=FIN_EVERY + 2) as ypool, \
                tc.tile_pool(name="psum", bufs=8, space="PSUM") as psum:
            bt = cpool.tile([P, OUT_DIM], _f32, tag="b")
            it = cpool.tile([P, P, max_gch], _bf16, tag="iotar")
            dt_ = cpool.tile([P, NG], _f32, tag="deg")
            norm = cpool.tile([P, NG], _f32, tag="norm")

            s_all = cpool.tile([P, NG], _f32, tag="sall")
            ls_all = cpool.tile([P, NG], _f32, tag="lsall")

            yr_tiles = {}
            nrounds = len(meta["rounds"])
            for ri, rnd in enumerate(meta["rounds"]):
                gs = rnd["groups"]
                rg = len(gs)
                nch = rnd["nch"]
                nidx_cols = sum(rnd["q_numidx"]) // 16
                ixt = mpool.tile([P, nidx_cols], _i16, tag="ix")
                nc.sync.dma_start(
                    out=ixt[:],
                    in_=gidx[:, rnd["idx_off"]:rnd["idx_off"] + nidx_cols])
                nldt = rnd["nldt"]
                ldt = mpool.tile([P, nldt], _bf16, tag="ld")
                nc.scalar.dma_start(
                    out=ldt[:],
                    in_=ldst[:, rnd["ch_off"]:rnd["ch_off"] + nldt])

                gt = gpool.tile([P, nch, OUT_DIM], _bf16, tag="gt")
                icol = 0
                for q in range(NT):
                    for par in range(2):
                        nq = rnd["q_numidx"][q * 2 + par]
                        if nq == 0:
                            continue
                        co = rnd["q_choff"][q * 2 + par]
                        _dma_gather_half(
                            nc.gpsimd,
                            out_ap=gt[:, co:co + nq // P, :],
                            in_ap=tabs[q][:, par * OUT_DIM:(par + 1) * OUT_DIM],
                            idxs_ap=ixt[:, icol:icol + nq // 16],
                            num_idxs=nq,
                            elem_size=OUT_DIM,
                            elem_step=2 * OUT_DIM,
                            single_packet=False,
                        )
                        icol += nq // 16
                if ri == 0:
                    # emitted after round 0's gathers so they don't delay
                    # them; it[p, m, c] = m (0..127, exact in bf16)
                    nc.gpsimd.iota(
                        out=it[:, :, :], pattern=[[1, P], [0, max_gch]],
                        base=0, channel_multiplier=0,
                        allow_small_or_imprecise_dtypes=True)
                    nc.scalar.dma_start(out=bt[:], in_=brep[:, :])
                    nc.scalar.dma_start(out=dt_[:], in_=ideg[:, :])
                    nc.vector.tensor_scalar_max(out=dt_[:], in0=dt_[:],
                                                scalar1=1.0)
                    nc.vector.reciprocal(out=dt_[:], in_=dt_[:])
                    nc.scalar.sqrt(out=norm[:], in_=dt_[:])

                xr = epool.tile([P, ROUND_G, OUT_DIM], _f32, tag="xr")
                for i, g in enumerate(gs):
                    ldt0, ngch = rnd["gldt"][g]
                    gtcols = rnd["ggt"][g]
                    if ngch == 0:
                        # group with no in-edges on any core
                        nc.vector.memset(xr[:, i, :], 0.0)
                        continue
                    # one-hot, chunk-last: oh[k, m, c] = (ldst[k,col_c]==m)
                    # all APs keep a packed last dim -> DVE 2x mode
                    oh = opool.tile([P, P, max_gch], _bf16, tag="oh")
                    nc.vector.tensor_tensor(
                        out=oh[:, :, 0:ngch],
                        in0=_expand_mid(ldt[:, ldt0:ldt0 + ngch], P),
                        in1=it[:, :, 0:ngch],
                        op=mybir.AluOpType.is_equal)
                    acc = psum.tile([P, OUT_DIM], _f32, tag="acc")
                    for k, cg in enumerate(gtcols):
                        nc.tensor.matmul(
                            acc[:], oh[:, :, k], gt[:, cg, :],
                            start=(k == 0), stop=(k == ngch - 1))
                    nc.scalar.activation(
                        out=xr[:, i, :], in_=acc[:],
                        func=mybir.ActivationFunctionType.Identity,
                        scale=norm[:, g:g + 1])

                g0 = gs[0]
                # batched epilogue for the round's rg groups
                nc.vector.tensor_tensor(
                    out=xr[:, :rg, :], in0=xr[:, :rg, :],
                    in1=_expand_mid(bt[:, :], rg),
                    op=mybir.AluOpType.add)
                nmx = epool.tile([P, ROUND_G], _f32, tag="nmx")
                nc.vector.tensor_reduce(out=nmx[:, :rg], in_=xr[:, :rg, :],
                                        axis=mybir.AxisListType.X,
                                        op=mybir.AluOpType.max,
                                        negate=True)
                yr = ypool.tile([P, ROUND_G, OUT_DIM], _bf16, tag="yr")
                yr_tiles[ri] = yr
                nc.vector.tensor_tensor(
                    out=yr[:, :rg, :], in0=xr[:, :rg, :],
                    in1=_expand_last(nmx[:, :rg], OUT_DIM),
                    op=mybir.AluOpType.add)
                e = epool.tile([P, ROUND_G, OUT_DIM], _f32, tag="e")
                for i, g in enumerate(gs):
                    nc.scalar.activation(
                        out=e[:, i, :], in_=yr[:, i, :],
                        func=mybir.ActivationFunctionType.Exp,
                        accum_out=s_all[:, g:g + 1])
                # clustered log_softmax finalize: one Ln per FIN_EVERY rounds
                # keeps Exp<->Ln act-table swaps off the per-round path
                if (ri + 1) % FIN_EVERY == 0 or ri == nrounds - 1:
                    r_lo = (ri // FIN_EVERY) * FIN_EVERY
                    glo = meta["rounds"][r_lo]["groups"][0]
                    nc.scalar.activation(
                        out=ls_all[:, glo:g0 + rg],
                        in_=s_all[:, glo:g0 + rg],
                        func=mybir.ActivationFunctionType.Ln)
                    for rj in range(r_lo, ri + 1):
                        gsj = meta["rounds"][rj]["groups"]
                        gj0, rgj = gsj[0], len(gsj)
                        yj = yr_tiles.pop(rj)
                        nc.vector.tensor_tensor(
                            out=yj[:, :rgj, :], in0=yj[:, :rgj, :],
                            in1=_expand_last(ls_all[:, gj0:gj0 + rgj],
                                             OUT_DIM),
                            op=mybir.AluOpType.subtract)
                        nc.scalar.dma_start(out=out[:, gj0:gj0 + rgj, :],
                                            in_=yj[:, :rgj, :])
    nc.compile()
    return nc


# ------------------------------------------------------------- host prep
def _wrap_idx16(flat):
    """int16 index list -> [128, len/16] wrapped layout (16-partition groups,
    replicated across the 8 gpsimd cores)."""
    n = len(flat)
    s = n // 16
    arr = np.empty((P, s), dtype=np.int16)
    blk = flat.reshape(s, 16).T  # [16, s]
    for grp in range(8):
        arr[grp * 16:(grp + 1) * 16, :] = blk
    return arr


def prepare(h, W, b, edges):
    h = np.asarray(h, dtype=np.float32)
    W = np.asarray(W, dtype=np.float32)
    b = np.asarray(b, dtype=np.float32)
    src = np.asarray(edges[0], dtype=np.int64)
    dst = np.asarray(edges[1], dtype=np.int64)

    out_deg = np.bincount(src, minlength=N_NODES).astype(np.float32)
    in_deg = np.bincount(dst, minlength=N_NODES).astype(np.float32)

    # global m-table row for each src node (padded per-core layout), then
    # paired-row coordinates: pair index + parity -> (sub-table, class)
    score = src // G
    mrow = score * GPAD + (src - score * G)
    pair = mrow >> 1
    par = mrow & 1
    qtab = pair // TROWS
    lrow = (pair - qtab * TROWS).astype(np.int16)
    cls = qtab * 2 + par

    dcore = dst // G
    dloc = dst - dcore * G
    grp = dloc // P
    ldst_v = (dloc - grp * P).astype(np.float32)

    # bucket = (dst-core, group, class)
    bucket = (dcore * NG + grp) * NCLS + cls
    order = np.argsort(bucket, kind="stable")
    bucket_s = bucket[order]
    lrow_s = lrow[order]
    ldst_s = ldst_v[order]

    nbuck = NCORES * NG * NCLS
    counts = np.bincount(bucket_s, minlength=nbuck).reshape(NCORES, NG, NCLS)
    starts = np.zeros(nbuck + 1, dtype=np.int64)
    np.cumsum(counts.reshape(-1), out=starts[1:])

    # uniform capacity per (group, class): max over cores, ceil to 16 (the
    # gather index wrap granularity). Groups pack back-to-back within a
    # (round, class) gather; a 128-slot chunk straddling two groups is
    # matmul'd once per group, each with its own ldst column (foreign and
    # padded slots carry PAD_LDST so they contribute zero).
    cap = counts.max(axis=0)                      # [NG, NCLS]
    cap16 = ((cap + 15) // 16) * 16               # [NG, NCLS]

    rounds = []
    idx_off = 0
    ch_off = 0
    r0 = 0
    for rsz in ROUND_SIZES:
        gs = list(range(r0, r0 + rsz))
        r0 += rsz
        q_numidx, q_choff = [], []
        slot0 = {}           # (g, c) -> slot offset within round's gather tile
        cursor = 0           # slots
        for c in range(NCLS):
            q_choff.append(cursor // P)
            base = cursor
            for g in gs:
                slot0[(g, c)] = cursor
                cursor += int(cap16[g, c])
            # pad class total to full chunks
            cursor = ((cursor + P - 1) // P) * P
            q_numidx.append(cursor - base)
        nch = cursor // P
        # per-group chunk columns (incl. shared boundary chunks) + one ldst
        # column per (g, chunk) pair
        gldt, ggt, lcol = {}, {}, {}
        lcur = 0
        for g in gs:
            cols = []
            for c in range(NCLS):
                s0c = slot0[(g, c)]
                s1c = s0c + int(cap16[g, c])
                if s1c > s0c:
                    cols.extend(range(s0c // P, (s1c - 1) // P + 1))
            gldt[g] = (lcur, len(cols))
            ggt[g] = cols
            for k, col in enumerate(cols):
                lcol[(g, col)] = lcur + k
            lcur += len(cols)
        rounds.append(dict(groups=gs, q_numidx=q_numidx, q_choff=q_choff,
                           nch=nch, nldt=lcur, idx_off=idx_off,
                           ch_off=ch_off, gldt=gldt, ggt=ggt, slot0=slot0,
                           lcol=lcol))
        idx_off += sum(q_numidx) // 16
        ch_off += lcur
    max_gch = max(rnd["gldt"][g][1] for rnd in rounds for g in rnd["groups"])
    meta = dict(rounds=rounds, tot_idx_cols=idx_off, tot_chunks=ch_off,
                max_gch=max_gch)

    # per-core gidx / ldst arrays
    gidx_cores = []
    ldst_cores = []
    for c0 in range(NCORES):
        flat_idx = np.zeros(idx_off * 16, dtype=np.int16)
        ld = np.full((P, ch_off), PAD_LDST, dtype=np.float32)
        for rnd in rounds:
            base16 = rnd["idx_off"] * 16
            for c in range(NCLS):
                for g in rnd["groups"]:
                    bid = (c0 * NG + g) * NCLS + c
                    s0, s1 = starts[bid], starts[bid + 1]
                    n = s1 - s0
                    pos = base16 + rnd["slot0"][(g, c)]
                    flat_idx[pos:pos + n] = lrow_s[s0:s1]
                    # ldst: slot j sits in chunk col (slot//P), lane slot%P,
                    # and uses group g's ldst column for that chunk
                    sl = rnd["slot0"][(g, c)] + np.arange(n)
                    lc = np.array([rnd["lcol"][(g, int(s) // P)]
                                   for s in sl], dtype=np.int64)
                    ld[sl % P, rnd["ch_off"] + lc] = ldst_s[s0:s1]
        gidx_cores.append(_wrap_idx16(flat_idx))
        ldst_cores.append(ld.astype(ml_dtypes.bfloat16))

    # degree tiles [128, NG] (partition = node % 128 within group)
    def deg_tile(deg):
        tiles = []
        for c in range(NCORES):
            d = np.ones(GPAD, dtype=np.float32)
            d[:G] = deg[c * G:(c + 1) * G]
            tiles.append(d.reshape(NG, P).T.copy())
        return tiles

    odeg_tiles = deg_tile(out_deg)
    ideg_tiles = deg_tile(in_deg)

    hT_cores = []
    for c in range(NCORES):
        hp = np.zeros((GPAD, IN_DIM), dtype=np.float32)
        hp[:G] = h[c * G:(c + 1) * G]
        # [2, 128, GPAD]: k-halves, contiguous along nodes for wide DMAs
        ht = np.ascontiguousarray(hp.T.reshape(2, P, GPAD))
        hT_cores.append(ht.astype(ml_dtypes.bfloat16))

    wt = np.ascontiguousarray(W.reshape(2, P, OUT_DIM)).astype(
        ml_dtypes.bfloat16)
    brep = np.broadcast_to(b, (P, OUT_DIM)).copy()

    return dict(meta=meta, gidx=gidx_cores, ldst=ldst_cores,
                odeg=odeg_tiles, ideg=ideg_tiles, hT=hT_cores,
                W=wt, brep=brep)


_cache = {}


def _get_programs(meta):
    if "a" not in _cache:
        _cache["a"] = build_launch_a()
    if "b" not in _cache:
        _cache["b"] = build_launch_b(meta)
    return _cache["a"], _cache["b"]


def run_launch_a(nc_a, prep):
    in_maps = [{"hT": prep["hT"][c], "W": prep["W"], "odeg": prep["odeg"][c]}
               for c in range(NCORES)]
    res = run_bass_kernel_spmd(nc_a, in_maps, list(range(NCORES)))
    # m[p, g*64+f] -> rows (g*128+p, f)
    shards = []
    for r in res.results:
        md = np.asarray(r["m"]).reshape(P, NG, OUT_DIM)
        shards.append(md.transpose(1, 0, 2).reshape(GPAD, OUT_DIM))
    return shards


def run_launch_b(nc_b, prep, m_shards):
    m_full = np.concatenate(m_shards, axis=0)  # [NCORES*GPAD, 64] bf16
    mp = m_full.reshape(NPAIR, 2 * OUT_DIM)    # paired rows, 256 B stride
    tabs = {f"t{q}": np.ascontiguousarray(mp[q * TROWS:(q + 1) * TROWS])
            for q in range(NT)}
    in_maps = [dict(tabs, gidx=prep["gidx"][c], ldst=prep["ldst"][c],
                    ideg=prep["ideg"][c], brep=prep["brep"])
               for c in range(NCORES)]
    res = run_bass_kernel_spmd(nc_b, in_maps, list(range(NCORES)))
    outs = []
    for r in res.results:
        od = np.asarray(r["out"])  # [P, NG, 64]
        outs.append(od.transpose(1, 0, 2).reshape(GPAD, OUT_DIM)[:G])
    return np.concatenate(outs, axis=0)


def kernel(h, W, b, edges):
    prep = prepare(h, W, b, edges)
    nc_a, nc_b = _get_programs(prep["meta"])
    m_shards = run_launch_a(nc_a, prep)
    out = run_launch_b(nc_b, prep, m_shards)
    return out.astype(np.float32)


# revision 59
# speedup vs baseline: 1.0066x; 1.0066x over previous
"""GraphConv (DGL norm='both') + log_softmax on 8 Trainium2 NeuronCores.

Strategy (per sharding hint): partition nodes across the 8 cores by range.
  Launch A (per core): project its 12500-node slice m = (h @ W) * out_deg^-1/2
  in bf16 (PE bf16, PSUM f32 accumulate).
  Host: concatenate the 8 projected shards into a replicated gather table,
  viewed as PAIRED rows [50176, 128] bf16 so the table row stride is 256 B
  (DMA descriptor encoding granularity) while each gather moves only the
  needed 128-B half-row (the pair parity selects a 64-col offset).
  Launch B (per core): for its 12500 dst nodes, gather m[src] half-rows for
  all in-edges (dma_gather, edges pre-sorted by dst group), segment-sum via
  one-hot matmuls accumulating in PSUM, then norm/bias/log_softmax.

Degrees and the sorted/padded edge metadata are sharding-prep computed on the
host (numpy); all FLOPs on h/W/b/m (projection, normalization, aggregation,
softmax) run on device.
"""

import contextlib

import numpy as np
import ml_dtypes

import concourse.bass as bass
import concourse.bacc as bacc
import concourse.mybir as mybir
import concourse.tile as tile
from concourse.bass import AP
from concourse.bass_utils import run_bass_kernel_spmd

P = 128
N_NODES = 100000
N_EDGES = 3200000
IN_DIM = 256
OUT_DIM = 64
NCORES = 8
G = N_NODES // NCORES            # 12500 nodes per core
NG = (G + P - 1) // P            # 98 groups of 128 dst nodes (last has 84)
GPAD = NG * P                    # 12544
NPAIR = (NCORES * GPAD) // 2     # 50176 paired table rows
NT = 2                           # sub-tables (int16 index limit)
TROWS = NPAIR // NT              # 25088 rows per sub-table
NCLS = NT * 2                    # gather classes: (sub-table, parity)
ROUND_G = 8                      # max dst groups per gather round
# tapered round sizes: short first round starts compute early; short last
# rounds keep the post-gather drain chain small
ROUND_SIZES = [4] + [8] * 11 + [3, 2, 1]
assert sum(ROUND_SIZES) == NG
assert max(ROUND_SIZES) <= ROUND_G
FIN_EVERY = 4                    # rounds per log_softmax finalize batch
HBLK = 16                        # dst groups per hT load in launch A
PAD_LDST = 200.0                 # local-dst for padded edges (>127, exact bf16)

_f32 = mybir.dt.float32
_bf16 = mybir.dt.bfloat16
_i16 = mybir.dt.int16


def _expand_mid(ap, n):
    """[P, C] AP -> [P, n, C] AP repeating each partition row n times
    (middle broadcast keeps the last dim packed, so DVE 2x mode applies)."""
    (ps, pc), (cs, cc) = ap.ap[0], ap.ap[1]
    return AP(ap.tensor, ap.offset, [[ps, pc], [0, n], [cs, cc]])


def _expand_last(ap, n):
    """[P, C] AP -> [P, C, n] AP repeating each element n times along a new
    innermost (stride-0) dim."""
    (ps, pc), (cs, cc) = ap.ap[0], ap.ap[1]
    return AP(ap.tensor, ap.offset, [[ps, pc], [cs, cc], [0, n]])


def _dma_gather_half(eng, out_ap, in_ap, idxs_ap, num_idxs, elem_size,
                     elem_step, single_packet=False):
    """dma_gather with a sub-256B payload (row stride must stay 256B-aligned:
    elem_step * dtype_size % 256 == 0). Same IR as bass's dma_gather helper,
    minus its payload-granularity assert (the HW descriptor only constrains
    the stride; the payload is free-form)."""
    stride_bytes = elem_step * mybir.dt.size(in_ap.dtype)
    assert stride_bytes % 256 == 0
    assert in_ap.ap[0][0] == elem_step
    return eng.add_instruction(
        mybir.InstDMAGatherAnt(
            name=eng.bass.get_next_instruction_name(),
            ins=[*eng.lower_ap_dma(in_ap, for_custom_bir_dma=True),
                 eng.lower_ap(idxs_ap),
                 eng.lower_val_access(eng.to_reg(num_idxs))],
            outs=[eng.lower_ap(out_ap)],
            transpose=False,
            num_idxs=num_idxs,
            elem_size=elem_size,
            stride_bytes_256=stride_bytes // 256,
            gen_mode=0,
            single_packet=single_packet,
            queue_num=0,
            sbuf_tokens_per_rank=0,
            sbuf_free_dim_per_rank=0,
            sbuf_free_dim_pad_per_rank=0,
            sbuf_byte_offset=0,
        ))


# ---------------------------------------------------------------- launch A
def build_launch_a(repeat=1):
    nc = bacc.Bacc("TRN2", target_bir_lowering=False, debug=False,
                   num_devices=NCORES)
    hT = nc.dram_tensor("hT", [2, P, GPAD], _bf16, kind="ExternalInput")
    W = nc.dram_tensor("W", [2, P, OUT_DIM], _bf16, kind="ExternalInput")
    odeg = nc.dram_tensor("odeg", [P, NG], _f32, kind="ExternalInput")
    # partition-major projected features: m[p, g*64+f] = m_row(g*128+p, f)
    m = nc.dram_tensor("m", [P, NG * OUT_DIM], _bf16, kind="ExternalOutput")

    with tile.TileContext(nc) as tc:
        loop = tc.For_i(0, repeat, 1) if repeat > 1 \
            else contextlib.nullcontext()
        with loop, \
                tc.tile_pool(name="const", bufs=1) as cpool, \
                tc.tile_pool(name="hblk", bufs=5) as hpool, \
                tc.tile_pool(name="mstage", bufs=4) as mpool, \
                tc.tile_pool(name="psum", bufs=8, space="PSUM") as psum:
            w0 = cpool.tile([P, OUT_DIM], _bf16, tag="w0")
            w1 = cpool.tile([P, OUT_DIM], _bf16, tag="w1")
            nc.scalar.dma_start(out=w0[:], in_=W[0, :, :])
            nc.scalar.dma_start(out=w1[:], in_=W[1, :, :])

            dt_ = cpool.tile([P, NG], _f32, tag="deg")
            norm = cpool.tile([P, NG], _f32, tag="norm")
            nc.scalar.dma_start(out=dt_[:], in_=odeg[:, :])
            nc.vector.tensor_scalar_max(out=dt_[:], in0=dt_[:], scalar1=1.0)
            nc.vector.reciprocal(out=dt_[:], in_=dt_[:])
            nc.scalar.sqrt(out=norm[:], in_=dt_[:])

            for g0 in range(0, NG, HBLK):
                nb = min(HBLK, NG - g0)
                # both k-halves in one DMA: dram-side AP iterates (p, half,
                # node) to match the SBUF tile's (partition, half, node)
                lh = hpool.tile([P, 2, HBLK * P], _bf16, tag="lh")
                src = AP(hT[0, :, :].tensor, g0 * P,
                         [[GPAD, P], [P * GPAD, 2], [1, nb * P]])
                nc.sync.dma_start(out=lh[:, :, :nb * P], in_=src)
                ms = mpool.tile([P, HBLK, OUT_DIM], _bf16, tag="ms")
                # 8 groups per PSUM bank; norm scaling batched on DVE
                for j0 in range(0, nb, 8):
                    nj = min(8, nb - j0)
                    acc8 = psum.tile([P, 8, OUT_DIM], _f32, tag="acc8")
                    for j in range(j0, j0 + nj):
                        nc.tensor.matmul(acc8[:, j - j0, :],
                                         lh[:, 0, j * P:(j + 1) * P], w0[:],
                                         start=True, stop=False)
                        nc.tensor.matmul(acc8[:, j - j0, :],
                                         lh[:, 1, j * P:(j + 1) * P], w1[:],
                                         start=False, stop=True)
                    nc.vector.tensor_tensor(
                        out=ms[:, j0:j0 + nj, :], in0=acc8[:, :nj, :],
                        in1=_expand_last(norm[:, g0 + j0:g0 + j0 + nj],
                                         OUT_DIM),
                        op=mybir.AluOpType.mult)
                nc.gpsimd.dma_start(
                    out=m[:, g0 * OUT_DIM:(g0 + nb) * OUT_DIM],
                    in_=ms[:, :nb, :])
    nc.compile()
    return nc


# ---------------------------------------------------------------- launch B
def build_launch_b(meta, repeat=1):
    """meta["rounds"][i]:
      groups; q_numidx[NCLS]; q_choff[NCLS]; nch; idx_off; ch_off
      gldt: {g: (ldt_col_start, ngch)}   # ldst cols, group-major contiguous
      ggt:  {g: [gt_column, ...]}        # gather-tile column per oh chunk
    """
    nc = bacc.Bacc("TRN2", target_bir_lowering=False, debug=False,
                   num_devices=NCORES)
    tabs = [nc.dram_tensor(f"t{q}", [TROWS, 2 * OUT_DIM], _bf16,
                           kind="ExternalInput") for q in range(NT)]
    gidx = nc.dram_tensor("gidx", [P, meta["tot_idx_cols"]], _i16,
                          kind="ExternalInput")
    ldst = nc.dram_tensor("ldst", [P, meta["tot_chunks"]], _bf16,
                          kind="ExternalInput")
    max_gch = meta["max_gch"]
    ideg = nc.dram_tensor("ideg", [P, NG], _f32, kind="ExternalInput")
    brep = nc.dram_tensor("brep", [P, OUT_DIM], _f32, kind="ExternalInput")
    # partition-major output: out[p, g, f] = result(g*128+p, f)
    out = nc.dram_tensor("out", [P, NG, OUT_DIM], _bf16,
                         kind="ExternalOutput")

    with tile.TileContext(nc) as tc:
        loop = tc.For_i(0, repeat, 1) if repeat > 1 \
            else contextlib.nullcontext()
        with loop, \
                tc.tile_pool(name="const", bufs=1) as cpool, \
                tc.tile_pool(name="gath", bufs=2) as gpool, \
                tc.tile_pool(name="meta", bufs=5) as mpool, \
                tc.tile_pool(name="onehot", bufs=7) as opool, \
                tc.tile_pool(name="epi", bufs=4) as epool, \
                tc.tile_pool(name="yr", bufs=FIN_EVERY + 2) as ypool, \
                tc.tile_pool(name="psum", bufs=8, space="PSUM") as psum:
            bt = cpool.tile([P, OUT_DIM], _f32, tag="b")
            it = cpool.tile([P, P, max_gch], _bf16, tag="iotar")
            dt_ = cpool.tile([P, NG], _f32, tag="deg")
            norm = cpool.tile([P, NG], _f32, tag="norm")

            s_all = cpool.tile([P, NG], _f32, tag="sall")
            ls_all = cpool.tile([P, NG], _f32, tag="lsall")

            yr_tiles = {}
            nrounds = len(meta["rounds"])
            for ri, rnd in enumerate(meta["rounds"]):
                gs = rnd["groups"]
                rg = len(gs)
                nch = rnd["nch"]
                nidx_cols = sum(rnd["q_numidx"]) // 16
                ixt = mpool.tile([P, nidx_cols], _i16, tag="ix")
                nc.sync.dma_start(
                    out=ixt[:],
                    in_=gidx[:, rnd["idx_off"]:rnd["idx_off"] + nidx_cols])
                nldt = rnd["nldt"]
                ldt = mpool.tile([P, nldt], _bf16, tag="ld")
                nc.scalar.dma_start(
                    out=ldt[:],
                    in_=ldst[:, rnd["ch_off"]:rnd["ch_off"] + nldt])

                gt = gpool.tile([P, nch, OUT_DIM], _bf16, tag="gt")
                icol = 0
                for q in range(NT):
                    for par in range(2):
                        nq = rnd["q_numidx"][q * 2 + par]
                        if nq == 0:
                            continue
                        co = rnd["q_choff"][q * 2 + par]
                        _dma_gather_half(
                            nc.gpsimd,
                            out_ap=gt[:, co:co + nq // P, :],
                            in_ap=tabs[q][:, par * OUT_DIM:(par + 1) * OUT_DIM],
                            idxs_ap=ixt[:, icol:icol + nq // 16],
                            num_idxs=nq,
                            elem_size=OUT_DIM,
                            elem_step=2 * OUT_DIM,
                            single_packet=False,
                        )
                        icol += nq // 16
                if ri == 0:
                    # emitted after round 0's gathers so they don't delay
                    # them; it[p, m, c] = m (0..127, exact in bf16)
                    nc.gpsimd.iota(
                        out=it[:, :, :], pattern=[[1, P], [0, max_gch]],
                        base=0, channel_multiplier=0,
                        allow_small_or_imprecise_dtypes=True)
                    nc.scalar.dma_start(out=bt[:], in_=brep[:, :])
                    nc.scalar.dma_start(out=dt_[:], in_=ideg[:, :])
                    nc.vector.tensor_scalar_max(out=dt_[:], in0=dt_[:],
                                                scalar1=1.0)
                    nc.vector.reciprocal(out=dt_[:], in_=dt_[:])
                    nc.scalar.sqrt(out=norm[:], in_=dt_[:])

                xr = epool.tile([P, ROUND_G, OUT_DIM], _f32, tag="xr")
                for i, g in enumerate(gs):
                    ldt0, ngch = rnd["gldt"][g]
                    gtcols = rnd["ggt"][g]
                    if ngch == 0:
                        # group with no in-edges on any core
                        nc.vector.memset(xr[:, i, :], 0.0)
                        continue
                    # one-hot, chunk-last: oh[k, m, c] = (ldst[k,col_c]==m)
                    # all APs keep a packed last dim -> DVE 2x mode
                    oh = opool.tile([P, P, max_gch], _bf16, tag="oh")
                    nc.vector.tensor_tensor(
                        out=oh[:, :, 0:ngch],
                        in0=_expand_mid(ldt[:, ldt0:ldt0 + ngch], P),
                        in1=it[:, :, 0:ngch],
                        op=mybir.AluOpType.is_equal)
                    acc = psum.tile([P, OUT_DIM], _f32, tag="acc")
                    for k, cg in enumerate(gtcols):
                        nc.tensor.matmul(
                            acc[:], oh[:, :, k], gt[:, cg, :],
                            start=(k == 0), stop=(k == ngch - 1))
                    nc.scalar.activation(
                        out=xr[:, i, :], in_=acc[:],
                        func=mybir.ActivationFunctionType.Identity,
                        scale=norm[:, g:g + 1])

                g0 = gs[0]
                # batched epilogue for the round's rg groups
                nc.vector.tensor_tensor(
                    out=xr[:, :rg, :], in0=xr[:, :rg, :],
                    in1=_expand_mid(bt[:, :], rg),
                    op=mybir.AluOpType.add)
                nmx = epool.tile([P, ROUND_G], _f32, tag="nmx")
                nc.vector.tensor_reduce(out=nmx[:, :rg], in_=xr[:, :rg, :],
                                        axis=mybir.AxisListType.X,
                                        op=mybir.AluOpType.max,
                                        negate=True)
                yr = ypool.tile([P, ROUND_G, OUT_DIM], _bf16, tag="yr")
                yr_tiles[ri] = yr
                nc.vector.tensor_tensor(
                    out=yr[:, :rg, :], in0=xr[:, :rg, :],
                    in1=_expand_last(nmx[:, :rg], OUT_DIM),
                    op=mybir.AluOpType.add)
                e = epool.tile([P, ROUND_G, OUT_DIM], _f32, tag="e")
                for i, g in enumerate(gs):
                    nc.scalar.activation(
                        out=e[:, i, :], in_=yr[:, i, :],
                        func=mybir.ActivationFunctionType.Exp,
                        accum_out=s_all[:, g:g + 1])
                # clustered log_softmax finalize: one Ln per FIN_EVERY rounds
                # keeps Exp<->Ln act-table swaps off the per-round path
                if (ri + 1) % FIN_EVERY == 0 or ri == nrounds - 1:
                    r_lo = (ri // FIN_EVERY) * FIN_EVERY
                    glo = meta["rounds"][r_lo]["groups"][0]
                    nc.scalar.activation(
                        out=ls_all[:, glo:g0 + rg],
                        in_=s_all[:, glo:g0 + rg],
                        func=mybir.ActivationFunctionType.Ln)
                    for rj in range(r_lo, ri + 1):
                        gsj = meta["rounds"][rj]["groups"]
                        gj0, rgj = gsj[0], len(gsj)
                        yj = yr_tiles.pop(rj)
                        nc.vector.tensor_tensor(
                            out=yj[:, :rgj, :], in0=yj[:, :rgj, :],
                            in1=_expand_last(ls_all[:, gj0:gj0 + rgj],
                                             OUT_DIM),
                            op=mybir.AluOpType.subtract)
                        nc.scalar.dma_start(out=out[:, gj0:gj0 + rgj, :],
                                            in_=yj[:, :rgj, :])
    nc.compile()
    return nc


# ------------------------------------------------------------- host prep
def _wrap_idx16(flat):
    """int16 index list -> [128, len/16] wrapped layout (16-partition groups,
    replicated across the 8 gpsimd cores)."""
    n = len(flat)
    s = n // 16
    arr = np.empty((P, s), dtype=np.int16)
    blk = flat.reshape(s, 16).T  # [16, s]
    for grp in range(8):
        arr[grp * 16:(grp + 1) * 16, :] = blk
    return arr


def prepare(h, W, b, edges):
    h = np.asarray(h, dtype=np.float32)
    W = np.asarray(W, dtype=np.float32)
    b = np.asarray(b, dtype=np.float32)
    src = np.asarray(edges[0], dtype=np.int64)
    dst = np.asarray(edges[1], dtype=np.int64)

    out_deg = np.bincount(src, minlength=N_NODES).astype(np.float32)
    in_deg = np.bincount(dst, minlength=N_NODES).astype(np.float32)

    # global m-table row for each src node (padded per-core layout), then
    # paired-row coordinates: pair index + parity -> (sub-table, class)
    score = src // G
    mrow = score * GPAD + (src - score * G)
    pair = mrow >> 1
    par = mrow & 1
    qtab = pair // TROWS
    lrow = (pair - qtab * TROWS).astype(np.int16)
    cls = qtab * 2 + par

    dcore = dst // G
    dloc = dst - dcore * G
    grp = dloc // P
    ldst_v = (dloc - grp * P).astype(np.float32)

    # bucket = (dst-core, group, class)
    bucket = (dcore * NG + grp) * NCLS + cls
    order = np.argsort(bucket, kind="stable")
    bucket_s = bucket[order]
    lrow_s = lrow[order]
    ldst_s = ldst_v[order]

    nbuck = NCORES * NG * NCLS
    counts = np.bincount(bucket_s, minlength=nbuck).reshape(NCORES, NG, NCLS)
    starts = np.zeros(nbuck + 1, dtype=np.int64)
    np.cumsum(counts.reshape(-1), out=starts[1:])

    # uniform capacity per (group, class): max over cores, ceil to 16 (the
    # gather index wrap granularity). Groups pack back-to-back within a
    # (round, class) gather; a 128-slot chunk straddling two groups is
    # matmul'd once per group, each with its own ldst column (foreign and
    # padded slots carry PAD_LDST so they contribute zero).
    cap16 = counts.max(axis=0)                    # [NG, NCLS] (exact caps;
    # slot offsets need no alignment — only class totals pad to 128)

    rounds = []
    idx_off = 0
    ch_off = 0
    r0 = 0
    for rsz in ROUND_SIZES:
        gs = list(range(r0, r0 + rsz))
        r0 += rsz
        q_numidx, q_choff = [], []
        slot0 = {}           # (g, c) -> slot offset within round's gather tile
        cursor = 0           # slots
        for c in range(NCLS):
            q_choff.append(cursor // P)
            base = cursor
            for g in gs:
                slot0[(g, c)] = cursor
                cursor += int(cap16[g, c])
            # pad class total to full chunks
            cursor = ((cursor + P - 1) // P) * P
            q_numidx.append(cursor - base)
        nch = cursor // P
        # per-group chunk columns (incl. shared boundary chunks) + one ldst
        # column per (g, chunk) pair
        gldt, ggt, lcol = {}, {}, {}
        lcur = 0
        for g in gs:
            cols = []
            for c in range(NCLS):
                s0c = slot0[(g, c)]
                s1c = s0c + int(cap16[g, c])
                if s1c > s0c:
                    cols.extend(range(s0c // P, (s1c - 1) // P + 1))
            gldt[g] = (lcur, len(cols))
            ggt[g] = cols
            for k, col in enumerate(cols):
                lcol[(g, col)] = lcur + k
            lcur += len(cols)
        rounds.append(dict(groups=gs, q_numidx=q_numidx, q_choff=q_choff,
                           nch=nch, nldt=lcur, idx_off=idx_off,
                           ch_off=ch_off, gldt=gldt, ggt=ggt, slot0=slot0,
                           lcol=lcol))
        idx_off += sum(q_numidx) // 16
        ch_off += lcur
    max_gch = max(rnd["gldt"][g][1] for rnd in rounds for g in rnd["groups"])
    meta = dict(rounds=rounds, tot_idx_cols=idx_off, tot_chunks=ch_off,
                max_gch=max_gch)

    # per-core gidx / ldst arrays
    gidx_cores = []
    ldst_cores = []
    for c0 in range(NCORES):
        flat_idx = np.zeros(idx_off * 16, dtype=np.int16)
        ld = np.full((P, ch_off), PAD_LDST, dtype=np.float32)
        for rnd in rounds:
            base16 = rnd["idx_off"] * 16
            for c in range(NCLS):
                for g in rnd["groups"]:
                    bid = (c0 * NG + g) * NCLS + c
                    s0, s1 = starts[bid], starts[bid + 1]
                    n = s1 - s0
                    pos = base16 + rnd["slot0"][(g, c)]
                    flat_idx[pos:pos + n] = lrow_s[s0:s1]
                    # ldst: slot j sits in chunk col (slot//P), lane slot%P,
                    # and uses group g's ldst column for that chunk
                    sl = rnd["slot0"][(g, c)] + np.arange(n)
                    lc = np.array([rnd["lcol"][(g, int(s) // P)]
                                   for s in sl], dtype=np.int64)
                    ld[sl % P, rnd["ch_off"] + lc] = ldst_s[s0:s1]
        gidx_cores.append(_wrap_idx16(flat_idx))
        ldst_cores.append(ld.astype(ml_dtypes.bfloat16))

    # degree tiles [128, NG] (partition = node % 128 within group)
    def deg_tile(deg):
        tiles = []
        for c in range(NCORES):
            d = np.ones(GPAD, dtype=np.float32)
            d[:G] = deg[c * G:(c + 1) * G]
            tiles.append(d.reshape(NG, P).T.copy())
        return tiles

    odeg_tiles = deg_tile(out_deg)
    ideg_tiles = deg_tile(in_deg)

    hT_cores = []
    for c in range(NCORES):
        hp = np.zeros((GPAD, IN_DIM), dtype=np.float32)
        hp[:G] = h[c * G:(c + 1) * G]
        # [2, 128, GPAD]: k-halves, contiguous along nodes for wide DMAs
        ht = np.ascontiguousarray(hp.T.reshape(2, P, GPAD))
        hT_cores.append(ht.astype(ml_dtypes.bfloat16))

    wt = np.ascontiguousarray(W.reshape(2, P, OUT_DIM)).astype(
        ml_dtypes.bfloat16)
    brep = np.broadcast_to(b, (P, OUT_DIM)).copy()

    return dict(meta=meta, gidx=gidx_cores, ldst=ldst_cores,
                odeg=odeg_tiles, ideg=ideg_tiles, hT=hT_cores,
                W=wt, brep=brep)


_cache = {}


def _get_programs(meta):
    if "a" not in _cache:
        _cache["a"] = build_launch_a()
    if "b" not in _cache:
        _cache["b"] = build_launch_b(meta)
    return _cache["a"], _cache["b"]


def run_launch_a(nc_a, prep):
    in_maps = [{"hT": prep["hT"][c], "W": prep["W"], "odeg": prep["odeg"][c]}
               for c in range(NCORES)]
    res = run_bass_kernel_spmd(nc_a, in_maps, list(range(NCORES)))
    # m[p, g*64+f] -> rows (g*128+p, f)
    shards = []
    for r in res.results:
        md = np.asarray(r["m"]).reshape(P, NG, OUT_DIM)
        shards.append(md.transpose(1, 0, 2).reshape(GPAD, OUT_DIM))
    return shards


def run_launch_b(nc_b, prep, m_shards):
    m_full = np.concatenate(m_shards, axis=0)  # [NCORES*GPAD, 64] bf16
    mp = m_full.reshape(NPAIR, 2 * OUT_DIM)    # paired rows, 256 B stride
    tabs = {f"t{q}": np.ascontiguousarray(mp[q * TROWS:(q + 1) * TROWS])
            for q in range(NT)}
    in_maps = [dict(tabs, gidx=prep["gidx"][c], ldst=prep["ldst"][c],
                    ideg=prep["ideg"][c], brep=prep["brep"])
               for c in range(NCORES)]
    res = run_bass_kernel_spmd(nc_b, in_maps, list(range(NCORES)))
    outs = []
    for r in res.results:
        od = np.asarray(r["out"])  # [P, NG, 64]
        outs.append(od.transpose(1, 0, 2).reshape(GPAD, OUT_DIM)[:G])
    return np.concatenate(outs, axis=0)


def kernel(h, W, b, edges):
    prep = prepare(h, W, b, edges)
    nc_a, nc_b = _get_programs(prep["meta"])
    m_shards = run_launch_a(nc_a, prep)
    out = run_launch_b(nc_b, prep, m_shards)
    return out.astype(np.float32)
